# revision 10
# baseline (speedup 1.0000x reference)
"""Channel-attention (XCA-style) Trainium2 kernel, 8-core SPMD. v2

Sharding: spatial row-bands (32 rows/core + 1-row halo), both batches on
every core. Cross-core coupling is the per-(batch,head) q@k.T block and
q/k squared norms, all-reduced as ~41KB fp32 per batch.

v2 structure (vs baseline):
  - proj+attn (Peff) is FOLDED into the 3x3 conv weights per batch, so
    the conv emits the final output directly; final_phase is gone.
  - conv K-packing: v1c tile pairs ch128-191@dy with ch128-191@dy+1 so
    tap pairs (-1,dx)+(0,dx) run as one K=128 matmul; 30 MMs/NB chunk.
  - gram: only the 48x48 q@k.T block per head on PE (N=48 matmuls);
    channel sq-norms via ScalarE Square+accum_out on untransposed q/k
    (2 position-half partials per ct, summed after the AllReduce).
  - stats path (extract/post) on Scalar+GpSimd+PE only; 1/x and
    1/sqrt(x) as exp(-a*ln(x)) on ScalarE. DVE does only the depthwise.
  - queues: sync = loads/transposes/AR staging, scalar = psum evacs +
    output stores, gpsimd = memsets/collective/small copies.
"""
import os
import sys

sys.path.insert(0, '/opt/trn_rl_repo')

import numpy as np
import ml_dtypes

import concourse.bass as bass
import concourse.bacc as bacc
import concourse.tile as tile
import concourse.mybir as mybir
from concourse.bass_utils import run_bass_kernel_spmd

BF16 = mybir.dt.bfloat16
F32 = mybir.dt.float32
ADD = mybir.AluOpType.add
MULT = mybir.AluOpType.mult
EXP = mybir.ActivationFunctionType.Exp
LN = mybir.ActivationFunctionType.Ln
SQUARE = mybir.ActivationFunctionType.Square

N_CORES = 8
DIM = 192
HEADS = 4
HD = DIM // HEADS            # 48
UD = 2 * HD                  # 96 channels per head in u=[q_h;k_h] order
B = 2
H = 256
W = 256
ROWS = H // N_CORES          # 32 output rows per core
RIN = ROWS + 2               # input band rows (halo)
WG = W + 2                   # guarded width
NI = ROWS * W                # 8192 interior positions per batch
NF = RIN * W                 # 8704 band positions per batch
NB = 512                     # matmul N-chunk
GRP = 512                    # streaming load group (cols)
TAPS = [(dy, dx) for dy in (-1, 0, 1) for dx in (-1, 0, 1)]
PAIR_DXS = (-1, 0, 1)
NPART = 4                    # norm partial sums per ct

# u-channel layout (after host perm): head h occupies [96h, 96h+96) with
# q first. qkt tile ct covers u-channels [128ct, 128ct+128).
# XBAR transpose sources starting at partition > 0 are limited to 32
# partitions -> split non-zero-base segments into 32-row chunks.


def _segs(raw):
    out = []
    for (ct, lo, hi) in raw:
        if lo == 0:
            out.append((ct, lo, hi))
        else:
            out += [(ct, s, min(s + 32, hi)) for s in range(lo, hi, 32)]
    return out


HEAD_SEGS = {
    0: _segs([(0, 0, 96)]),
    1: _segs([(0, 96, 128), (1, 0, 64)]),
    2: _segs([(1, 64, 128), (2, 0, 32)]),
    3: _segs([(2, 32, 128)]),
}
# q/k norm pieces as (ct, lo, hi) into the qn[:, ct-partial] columns
Q_PIECES = {0: [(0, 0, 48)], 1: [(0, 96, 128), (1, 0, 16)],
            2: [(1, 64, 112)], 3: [(2, 32, 80)]}
K_PIECES = {0: [(0, 48, 96)], 1: [(1, 16, 64)],
            2: [(1, 112, 128), (2, 0, 32)], 3: [(2, 80, 128)]}

LAST_RESULTS = None
_CACHED_NC = None


def _u_perm():
    p = []
    for h in range(HEADS):
        p += list(range(h * HD, (h + 1) * HD))
        p += list(range(DIM + h * HD, DIM + (h + 1) * HD))
    return np.array(p)


def _bf16(a):
    return np.ascontiguousarray(a.astype(ml_dtypes.bfloat16))


def build_nc():
    nc = bacc.Bacc("TRN2", target_bir_lowering=False, debug=False,
                   enable_asserts=False, num_devices=N_CORES)
    xb = nc.dram_tensor("xb", [B, DIM, NF], BF16, kind="ExternalInput").ap()
    yb = nc.dram_tensor("yb", [B, DIM, NF], BF16, kind="ExternalInput").ap()
    wqk = nc.dram_tensor("wqk", [DIM, 2 * DIM], BF16, kind="ExternalInput").ap()
    aqk = nc.dram_tensor("aqk", [3, 128, 9], F32, kind="ExternalInput").ap()
    wv = nc.dram_tensor("wv", [DIM, DIM], BF16, kind="ExternalInput").ap()
    # vdw in [c_out, in] layout per tap (lhsT of the Peff fold matmuls)
    wvdw = nc.dram_tensor("wvdw", [9, DIM, DIM], BF16, kind="ExternalInput").ap()
    projth = nc.dram_tensor("projth", [HEADS, HD, DIM], BF16, kind="ExternalInput").ap()
    tempt = nc.dram_tensor("tempt", [HD, HEADS], F32, kind="ExternalInput").ap()
    eye = nc.dram_tensor("eye", [HD, HD], F32, kind="ExternalInput").ap()
    out = nc.dram_tensor("out", [B, DIM, NI], F32, kind="ExternalOutput").ap()

    QC = HEADS * HD               # 192: offset of q-norm partial cols
    KC = QC + NPART * HEADS       # k-norm partial cols
    AW = KC + NPART * HEADS

    with tile.TileContext(nc) as tc:
        with tc.tile_pool(name="wpool", bufs=1) as wp, \
             tc.tile_pool(name="wfold", bufs=1) as wfp, \
             tc.tile_pool(name="xy", bufs=6) as xyp, \
             tc.tile_pool(name="mid", bufs=2) as midp, \
             tc.tile_pool(name="qk", bufs=1) as qkp, \
             tc.tile_pool(name="ut", bufs=2) as utp, \
             tc.tile_pool(name="vv", bufs=1) as vvp, \
             tc.tile_pool(name="small", bufs=1) as smp, \
             tc.tile_pool(name="small2", bufs=2) as smp2, \
             tc.tile_pool(name="ost", bufs=2) as ostp, \
             tc.tile_pool(name="psA", bufs=3, space="PSUM") as psA, \
             tc.tile_pool(name="psB", bufs=2, space="PSUM") as psB, \
             tc.tile_pool(name="psG", bufs=2, space="PSUM") as psG, \
             tc.tile_pool(name="psS", bufs=1, space="PSUM") as psS, \
             tc.tile_pool(name="dram", bufs=2, space="DRAM") as drp:

            # ---- persistent weights (gpsimd so sync stays clear) ----
            wqk_a = wp.tile([128, 2 * DIM], BF16, tag="wqk_a")
            wqk_b = wp.tile([64, 2 * DIM], BF16, tag="wqk_b")
            nc.gpsimd.dma_start(wqk_a[:], wqk[0:128, :])
            nc.gpsimd.dma_start(wqk_b[:], wqk[128:192, :])
            wv_a = wp.tile([128, DIM], BF16, tag="wv_a")
            wv_b = wp.tile([64, DIM], BF16, tag="wv_b")
            nc.gpsimd.dma_start(wv_a[:], wv[0:128, :])
            nc.gpsimd.dma_start(wv_b[:], wv[128:192, :])
            wvdw_ca = wp.tile([128, 9, DIM], BF16, tag="wvdw_ca")
            wvdw_cb = wp.tile([64, 9, DIM], BF16, tag="wvdw_cb")
            nc.gpsimd.dma_start(wvdw_ca[:], wvdw[:, 0:128, :].rearrange("t c i -> c t i"))
            nc.gpsimd.dma_start(wvdw_cb[:], wvdw[:, 128:192, :].rearrange("t c i -> c t i"))
            aqk_sb = wp.tile([128, 3, 9], F32, tag="aqk")
            nc.gpsimd.dma_start(aqk_sb[:], aqk.rearrange("c k t -> k c t"))
            projth_sb = wp.tile([HD, HEADS, DIM], BF16, tag="projth")
            nc.gpsimd.dma_start(projth_sb[:], projth.rearrange("h d f -> d h f"))
            tempt_sb = wp.tile([HD, HEADS], F32, tag="tempt")
            nc.gpsimd.dma_start(tempt_sb[:], tempt[:])
            eye_sb = wp.tile([HD, HD], F32, tag="eye")
            nc.gpsimd.dma_start(eye_sb[:], eye[:])
            ones_sb = wp.tile([1, HD], F32, tag="ones")
            nc.gpsimd.memset(ones_sb[:], 1.0)

            groups = [(g, min(g + GRP, NF)) for g in range(0, NF, GRP)]

            def qk_phase(b):
                """1x1 conv (PE) into guarded mid tiles + depthwise (DVE)."""
                qkt = []
                for ct in range(3):
                    mid = midp.tile([128, RIN, WG], BF16, tag="mid")
                    nc.gpsimd.memset(mid[:, :, 0:1], 0.0)
                    nc.gpsimd.memset(mid[:, :, WG - 1:WG], 0.0)
                    mcol = slice(ct * 128, (ct + 1) * 128)
                    for (g0, g1) in groups:
                        x_a = xyp.tile([128, GRP], BF16, tag="band_a")
                        x_b = xyp.tile([64, GRP], BF16, tag="band_b")
                        nc.sync.dma_start(x_a[:, 0:g1 - g0], xb[b, 0:128, g0:g1])
                        nc.sync.dma_start(x_b[:, 0:g1 - g0], xb[b, 128:192, g0:g1])
                        for nb in range(g0 // NB, g1 // NB):
                            ns = slice(nb * NB - g0, (nb + 1) * NB - g0)
                            ps = psA.tile([128, NB], F32, tag="psA")
                            nc.tensor.matmul(ps[:], wqk_a[:, mcol], x_a[:, ns],
                                             start=True, stop=False)
                            nc.tensor.matmul(ps[:], wqk_b[:, mcol], x_b[:, ns],
                                             start=False, stop=True)
                            r = nb * 2
                            nc.scalar.copy(mid[:, r:r + 2, 1:W + 1],
                                           ps[:].rearrange("p (r w) -> p r w", r=2))
                    # depthwise 3x3 split DVE/GpSimd, fused mul-add
                    qt = qkp.tile([128, ROWS, W], BF16, tag=f"qk{ct}")
                    qtB = qkp.tile([128, ROWS, W], BF16, tag="qtB")

                    def msrc(dy, dx):
                        return mid[:, 1 + dy:1 + ROWS + dy,
                                   1 + dx:1 + W + dx]

                    # gpsimd takes one tap (plain mul, no scratch);
                    # DVE does center + 7 fused mul-adds + the merge
                    t0 = TAPS.index((-1, -1))
                    nc.gpsimd.tensor_scalar_mul(
                        qtB[:], msrc(-1, -1), aqk_sb[:, ct, t0:t0 + 1])
                    nc.vector.tensor_scalar_mul(
                        qt[:], mid[:, 1:1 + ROWS, 1:1 + W], aqk_sb[:, ct, 4:5])
                    for (dy, dx) in [(-1, 0), (-1, 1), (0, -1), (0, 1),
                                     (1, -1), (1, 0), (1, 1)]:
                        t = TAPS.index((dy, dx))
                        nc.vector.scalar_tensor_tensor(
                            qt[:], msrc(dy, dx), aqk_sb[:, ct, t:t + 1],
                            qt[:], op0=MULT, op1=ADD)
                    nc.vector.tensor_tensor(qt[:], qtB[:], qt[:], op=ADD)
                    qkt.append(qt)
                return qkt

            def norms_phase(b, qkt):
                """squared channel norms of q/k on ScalarE: Square with
                accum_out, two position-half partials per ct (summed after
                the AllReduce). Scratch is an 8KB dedicated tile."""
                qn = smp2.tile([128, 3, NPART], F32, tag="qn")
                for ct in range(3):
                    for p in range(NPART):
                        scr = qkp.tile([128, ROWS // NPART, W], BF16,
                                       tag="sqscr")
                        nc.scalar.activation(
                            scr[:], qkt[ct][:, p * (ROWS // NPART):
                                            (p + 1) * (ROWS // NPART), :],
                            SQUARE, accum_out=qn[:, ct, p:p + 1])
                return qn

            def v1_phase(b):
                """1x1 conv for v into guarded v1a/v1b tiles."""
                v1a = vvp.tile([128, RIN, WG], BF16, tag="v1a")
                v1b = vvp.tile([64, RIN, WG], BF16, tag="v1b")
                for t_ in (v1a, v1b):
                    nc.gpsimd.memset(t_[:, :, 0:1], 0.0)
                    nc.gpsimd.memset(t_[:, :, WG - 1:WG], 0.0)
                for (g0, g1) in groups:
                    y_a = xyp.tile([128, GRP], BF16, tag="band_a")
                    y_b = xyp.tile([64, GRP], BF16, tag="band_b")
                    nc.sync.dma_start(y_a[:, 0:g1 - g0], yb[b, 0:128, g0:g1])
                    nc.sync.dma_start(y_b[:, 0:g1 - g0], yb[b, 128:192, g0:g1])
                    for nb in range(g0 // NB, g1 // NB):
                        ns = slice(nb * NB - g0, (nb + 1) * NB - g0)
                        ps = psA.tile([128, NB], F32, tag="psA", name="psv1a")
                        ps2 = psB.tile([64, NB], F32, tag="psB", name="psv1b")
                        nc.tensor.matmul(ps[:], wv_a[:, 0:128], y_a[:, ns],
                                         start=True, stop=False)
                        nc.tensor.matmul(ps[:], wv_b[:, 0:128], y_b[:, ns],
                                         start=False, stop=True)
                        nc.tensor.matmul(ps2[:], wv_a[:, 128:192], y_a[:, ns],
                                         start=True, stop=False)
                        nc.tensor.matmul(ps2[:], wv_b[:, 128:192], y_b[:, ns],
                                         start=False, stop=True)
                        r = nb * 2
                        nc.scalar.copy(v1a[:, r:r + 2, 1:W + 1],
                                       ps[:].rearrange("p (r w) -> p r w", r=2))
                        nc.scalar.copy(v1b[:, r:r + 2, 1:W + 1],
                                       ps2[:].rearrange("p (r w) -> p r w", r=2))
                return v1a, v1b

            def v1c_phase(v1b):
                # v1c: rows 0-63 = v1b; rows 64-127 = v1b shifted one band
                # row down, so one K=128 matmul covers taps dy and dy+1
                v1c = vvp.tile([128, RIN, WG], BF16, tag="v1c")
                nc.gpsimd.memset(v1c[64:128, RIN - 1:RIN, :], 0.0)
                nc.sync.dma_start(v1c[0:64, :, :], v1b[:, :, :])
                nc.sync.dma_start(v1c[64:128, 0:RIN - 1, :], v1b[:, 1:RIN, :])
                return v1c

            def gram_phase(b, qkt, gu):
                """DMA-transpose u per (head, band); accumulate the 48x48
                q@k.T block per head on PE. Position order inside ut is
                irrelevant (gram sums over positions)."""
                n_bands = NI // 2048
                for h in range(HEADS):
                    gps = psG.tile([HD, HD], F32, tag="psG")
                    for band in range(n_bands):
                        rsl = slice(band * 8, (band + 1) * 8)
                        ut = utp.tile([128, 16, UD], BF16, tag="ut")
                        off = 0
                        eng = nc.sync if (h * 4 + band) % 2 == 0 else nc.scalar
                        for (ct, lo, hi) in HEAD_SEGS[h]:
                            eng.dma_start_transpose(
                                ut[:, :, off:off + hi - lo],
                                qkt[ct][lo:hi, rsl, :])
                            off += hi - lo
                        for c in range(16):
                            nc.tensor.matmul(
                                gps[:], ut[:, c, 0:HD], ut[:, c, HD:UD],
                                start=(band == 0 and c == 0),
                                stop=(band == n_bands - 1 and c == 15))
                    nc.scalar.copy(gu[:, h, :], gps[:])

            def ar_phase(b, qn, gu):
                ar_in = drp.tile([HD, AW], F32, tag="ar_in")
                ar_out = drp.tile([HD, AW], F32, tag="ar_out")
                nc.sync.dma_start(ar_in[:, 0:QC],
                                  gu[:].rearrange("p h d -> p (h d)"))
                for h in range(HEADS):
                    for pieces, base in ((Q_PIECES[h], QC), (K_PIECES[h], KC)):
                        o = 0
                        for (ct, lo, hi) in pieces:
                            col = base + NPART * h
                            nc.sync.dma_start(
                                ar_in[o:o + hi - lo, col:col + NPART],
                                qn[lo:hi, ct, :])
                            o += hi - lo
                nc.gpsimd.collective_compute(
                    "AllReduce", ADD,
                    replica_groups=[list(range(N_CORES))],
                    ins=[ar_in.opt()], outs=[ar_out.opt()])
                gqk = smp2.tile([HD, HEADS, HD], F32, tag="gqk")
                qn2 = smp2.tile([HD, HEADS, NPART], F32, tag="qn2")
                kn2 = smp2.tile([HD, HEADS, NPART], F32, tag="kn2")
                nc.sync.dma_start(gqk[:].rearrange("p h d -> p (h d)"),
                                  ar_out[:, 0:QC])
                nc.sync.dma_start(qn2[:].rearrange("p h t -> p (h t)"),
                                  ar_out[:, QC:QC + NPART * HEADS])
                nc.sync.dma_start(kn2[:].rearrange("p h t -> p (h t)"),
                                  ar_out[:, KC:KC + NPART * HEADS])
                return gqk, qn2, kn2

            def post_phase(b, gqk, qn2, kn2):
                """softmax + Peff^T on Scalar/GpSimd/PE only (no DVE);
                1/sqrt(x), 1/x via exp(-a*ln(x)) on ScalarE."""
                qsum = smp.tile([HD, HEADS], F32, tag="qsum")
                ksum = smp.tile([HD, HEADS], F32, tag="ksum")
                qh_ = smp.tile([HD, HEADS, 2], F32, tag="qh_")
                kh_ = smp.tile([HD, HEADS, 2], F32, tag="kh_")
                nc.gpsimd.tensor_tensor(qh_[:], qn2[:, :, 0:2],
                                        qn2[:, :, 2:4], op=ADD)
                nc.gpsimd.tensor_tensor(kh_[:], kn2[:, :, 0:2],
                                        kn2[:, :, 2:4], op=ADD)
                nc.gpsimd.tensor_tensor(qsum[:], qh_[:, :, 0], qh_[:, :, 1],
                                        op=ADD)
                nc.gpsimd.tensor_tensor(ksum[:], kh_[:, :, 0], kh_[:, :, 1],
                                        op=ADD)
                lq = smp.tile([HD, HEADS], F32, tag="lq")
                invq = smp.tile([HD, HEADS], F32, tag="invq")
                nc.scalar.activation(lq[:], qsum[:], LN)
                nc.scalar.activation(invq[:], lq[:], EXP, scale=-0.5)
                nc.gpsimd.tensor_tensor(invq[:], invq[:], tempt_sb[:], op=MULT)
                # k sq-norms arrive as columns; PE-transpose to rows
                krow = smp.tile([1, HEADS, HD], F32, tag="krow")
                for h in range(HEADS):
                    pst = psS.tile([HD, HD], F32, tag="psS", name="pst")
                    nc.tensor.transpose(pst[0:1, :], ksum[:, h:h + 1], eye_sb[:])
                    nc.scalar.copy(krow[:, h, :], pst[0:1, :])
                lk = smp.tile([1, HEADS, HD], F32, tag="lk")
                invkr = smp.tile([1, HEADS, HD], F32, tag="invkr")
                nc.scalar.activation(lk[:], krow[:], LN)
                nc.scalar.activation(invkr[:], lk[:], EXP, scale=-0.5)
                mst = smp.tile([HD, HEADS, DIM], BF16, tag="mst")
                # batched by op type to avoid ACT table thrash
                bc = smp.tile([HD, HEADS, HD], F32, tag="bc")
                for h in range(HEADS):
                    bps = psS.tile([HD, HD], F32, tag="psS", name="bps")
                    nc.tensor.matmul(bps[:], ones_sb[:], invkr[:, h, :],
                                     start=True, stop=True)
                    nc.scalar.copy(bc[:, h, :], bps[:])
                lg = smp.tile([HD, HEADS, HD], F32, tag="lg")
                for h in range(HEADS):
                    nc.gpsimd.tensor_scalar_mul(lg[:, h, :], gqk[:, h, :],
                                                invq[:, h:h + 1])
                nc.gpsimd.tensor_tensor(lg[:], lg[:], bc[:], op=MULT)
                # logits = qhat.khat * temp, |logit| <= |temp|: no
                # max-subtraction needed; Exp + row-sum in one ACT
                ex = smp.tile([HD, HEADS, HD], F32, tag="ex")
                sm = smp.tile([HD, HEADS], F32, tag="sm")
                for h in range(HEADS):
                    nc.scalar.activation(ex[:, h, :], lg[:, h, :], EXP,
                                         accum_out=sm[:, h:h + 1])
                lsm = smp.tile([HD, HEADS], F32, tag="lsm")
                rs_ = smp.tile([HD, HEADS], F32, tag="rs_")
                nc.scalar.activation(lsm[:], sm[:], LN)
                nc.scalar.activation(rs_[:], lsm[:], EXP, scale=-1.0)
                at = smp.tile([HD, HEADS, HD], BF16, tag="at")
                for h in range(HEADS):
                    nc.gpsimd.tensor_scalar_mul(at[:, h, :], ex[:, h, :],
                                                rs_[:, h:h + 1])
                for h in range(HEADS):
                    mps = psS.tile([HD, DIM], F32, tag="psS", name="mps")
                    nc.tensor.matmul(mps[:], at[:, h, :], projth_sb[:, h, :],
                                     start=True, stop=True)
                    nc.scalar.copy(mst[:, h, :], mps[:])
                pe0 = smp.tile([128, DIM], BF16, tag="pe0")
                pe1 = smp.tile([64, DIM], BF16, tag="pe1")
                nc.sync.dma_start(pe0[0:48, :], mst[:, 0, :])
                nc.sync.dma_start(pe0[48:96, :], mst[:, 1, :])
                nc.sync.dma_start(pe0[96:128, :], mst[0:32, 2, :])
                nc.sync.dma_start(pe1[0:16, :], mst[32:48, 2, :])
                nc.sync.dma_start(pe1[16:64, :], mst[:, 3, :])
                return pe0, pe1

            def fold_phase(b, pe0, pe1):
                """W' = Peff @ Wdw staged as conv lhsT tiles:
                wfa    [128, 9, 192] : in-ch 0-127, all taps
                wfpair [128, 3, 192] : in-ch 128-191 stacked (-1,dx)|(0,dx)
                wfb1   [64, 3, 192]  : in-ch 128-191, tap (+1,dx)
                """
                wfa = wfp.tile([128, 9, DIM], BF16, tag="wfa")
                wfpair = wfp.tile([128, 3, DIM], BF16, tag="wfpair")
                wfb1 = wfp.tile([64, 3, DIM], BF16, tag="wfb1")
                for t in range(9):
                    psF = psS.tile([128, DIM], F32, tag="psS", name="psF")
                    nc.tensor.matmul(psF[:], wvdw_ca[:, t, 0:128], pe0[:],
                                     start=True, stop=False)
                    nc.tensor.matmul(psF[:], wvdw_cb[:, t, 0:128], pe1[:],
                                     start=False, stop=True)
                    nc.scalar.copy(wfa[:, t, :], psF[:])
                for j, dx in enumerate(PAIR_DXS):
                    t0 = TAPS.index((-1, dx))
                    t1 = TAPS.index((0, dx))
                    t2 = TAPS.index((1, dx))
                    psP = psS.tile([128, DIM], F32, tag="psS", name="psP")
                    nc.tensor.matmul(psP[0:64, :], wvdw_ca[:, t0, 128:192],
                                     pe0[:], start=True, stop=False)
                    nc.tensor.matmul(psP[0:64, :], wvdw_cb[:, t0, 128:192],
                                     pe1[:], start=False, stop=True)
                    nc.tensor.matmul(psP[64:128, :], wvdw_ca[:, t1, 128:192],
                                     pe0[:], start=True, stop=False,
                                     tile_position=(0, 64))
                    nc.tensor.matmul(psP[64:128, :], wvdw_cb[:, t1, 128:192],
                                     pe1[:], start=False, stop=True,
                                     tile_position=(0, 64))
                    nc.scalar.copy(wfpair[:, j, :], psP[:])
                    psL = psS.tile([64, DIM], F32, tag="psS", name="psL")
                    nc.tensor.matmul(psL[:], wvdw_ca[:, t2, 128:192], pe0[:],
                                     start=True, stop=False)
                    nc.tensor.matmul(psL[:], wvdw_cb[:, t2, 128:192], pe1[:],
                                     start=False, stop=True)
                    nc.scalar.copy(wfb1[:, j, :], psL[:])
                return wfa, wfpair, wfb1

            def vdw_out_phase(b, v1a, v1b, v1c, wfa, wfpair, wfb1,
                              chunk_cb=None):
                """folded 3x3 conv -> final output band, streamed to HBM.
                15 matmuls per psum chunk: 9 K=128 (in-ch 0-127), 3 K=128
                tap-pairs (v1c), 3 K=64 leftovers (v1b)."""
                for nb in range(NI // NB):
                    ns = slice(nb * NB, (nb + 1) * NB)
                    for (f0, f1) in ((0, 128), (128, 192)):
                        if f1 - f0 == 128:
                            psf = psA.tile([128, NB], F32, tag="psA",
                                           name="psfA")
                        else:
                            psf = psB.tile([64, NB], F32, tag="psB",
                                           name="psfB")
                        fsl = slice(f0, f1)
                        for t, (dy, dx) in enumerate(TAPS):
                            cs = slice(1 + dx, 1 + W + dx)
                            rs = slice(1 + nb * 2 + dy, 3 + nb * 2 + dy)
                            nc.tensor.matmul(
                                psf[:], wfa[:, t, fsl], v1a[:, rs, cs],
                                start=(t == 0), stop=False)
                        for j, dx in enumerate(PAIR_DXS):
                            cs = slice(1 + dx, 1 + W + dx)
                            rs = slice(nb * 2, 2 + nb * 2)      # dy=-1 rows
                            nc.tensor.matmul(
                                psf[:], wfpair[:, j, fsl], v1c[:, rs, cs],
                                start=False, stop=False)
                        for j, dx in enumerate(PAIR_DXS):
                            cs = slice(1 + dx, 1 + W + dx)
                            rs = slice(2 + nb * 2, 4 + nb * 2)  # dy=+1 rows
                            nc.tensor.matmul(
                                psf[:], wfb1[:, j, fsl], v1b[:, rs, cs],
                                start=False, stop=(j == 2))
                        ost = ostp.tile([f1 - f0, NB], F32,
                                        tag=f"ost{f1 - f0}")
                        nc.scalar.copy(ost[:], psf[:])
                        nc.scalar.dma_start(out[b, fsl, ns], ost[:])
                    if chunk_cb is not None:
                        chunk_cb(nb)

            # ================= global schedule =================
            qkt0 = qk_phase(0)
            v1a0, v1b0 = v1_phase(0)
            qkt1 = qk_phase(1)
            v1c0 = v1c_phase(v1b0)
            gu0 = smp2.tile([HD, HEADS, HD], F32, tag="gu")
            gram_phase(0, qkt0, gu0)
            qn0 = norms_phase(0, qkt0)
            gqk0, qn20, kn20 = ar_phase(0, qn0, gu0)
            pe00, pe10 = post_phase(0, gqk0, qn20, kn20)
            wf0 = fold_phase(0, pe00, pe10)

            # batch-1 gram rides under batch-0's conv: transposes/AR run
            # on sync/gpsimd while the PE streams conv matmuls
            state = {}

            def cb(nb):
                if nb == 8:
                    gu1 = smp2.tile([HD, HEADS, HD], F32, tag="gu")
                    gram_phase(1, qkt1, gu1)
                    qn1 = norms_phase(1, qkt1)
                    state['ar'] = ar_phase(1, qn1, gu1)

            vdw_out_phase(0, v1a0, v1b0, v1c0, *wf0, chunk_cb=cb)
            gqk1, qn21, kn21 = state['ar']
            pe01, pe11 = post_phase(1, gqk1, qn21, kn21)
            wf1 = fold_phase(1, pe01, pe11)
            v1a1, v1b1 = v1_phase(1)
            v1c1 = v1c_phase(v1b1)
            vdw_out_phase(1, v1a1, v1b1, v1c1, *wf1)

    nc.finalize()
    return nc


def _host_prep(inputs):
    x = np.asarray(inputs["x"], dtype=np.float32)
    y = np.asarray(inputs["y"], dtype=np.float32)
    qk_w = np.asarray(inputs["qk_w"], dtype=np.float32)[:, :, 0, 0]
    qk_dw = np.asarray(inputs["qk_dw_w"], dtype=np.float32)[:, 0]
    v_w = np.asarray(inputs["v_w"], dtype=np.float32)[:, :, 0, 0]
    v_dw = np.asarray(inputs["v_dw_w"], dtype=np.float32)
    proj = np.asarray(inputs["proj_w"], dtype=np.float32)[:, :, 0, 0]
    temp = np.asarray(inputs["temperature"], dtype=np.float32).reshape(HEADS)

    perm = _u_perm()
    wqk_l = _bf16(qk_w[perm].T)                              # [192, 384]
    aqk_t = np.ascontiguousarray(
        qk_dw[perm].reshape(3, 128, 9).astype(np.float32))
    wv_l = _bf16(v_w.T)                                      # [192, 192]
    # [9, c_out, in]: NOT transposed (lhsT of the fold matmuls)
    wvdw_l = _bf16(np.stack(
        [v_dw[:, :, dy + 1, dx + 1] for dy, dx in TAPS]))    # [9,192,192]
    projth = _bf16(np.stack(
        [proj[:, h * HD:(h + 1) * HD].T for h in range(HEADS)]))  # [4,48,192]
    tempt = np.ascontiguousarray(
        np.broadcast_to(temp[None, :], (HD, HEADS)).astype(np.float32))
    eye = np.eye(HD, dtype=np.float32)

    xp = np.pad(x, ((0, 0), (0, 0), (1, 1), (0, 0)))
    yp = np.pad(y, ((0, 0), (0, 0), (1, 1), (0, 0)))
    shared = dict(wqk=wqk_l, aqk=aqk_t, wv=wv_l, wvdw=wvdw_l,
                  projth=projth, tempt=tempt, eye=eye)
    in_maps = []
    for c in range(N_CORES):
        rs = slice(c * ROWS, c * ROWS + RIN)
        in_maps.append(dict(
            xb=_bf16(xp[:, :, rs]).reshape(B, DIM, NF),
            yb=_bf16(yp[:, :, rs]).reshape(B, DIM, NF),
            **shared))
    return in_maps


def kernel(**inputs):
    global LAST_RESULTS, _CACHED_NC
    in_maps = _host_prep(inputs)
    if _CACHED_NC is None:
        _CACHED_NC = build_nc()
    res = run_bass_kernel_spmd(
        _CACHED_NC, in_maps, core_ids=list(range(N_CORES)))
    LAST_RESULTS = res
    out = np.empty((B, DIM, H, W), np.float32)
    for c in range(N_CORES):
        band = res.results[c]["out"].reshape(B, DIM, ROWS, W)
        out[:, :, c * ROWS:(c + 1) * ROWS] = band
    return out
